# revision 10
# baseline (speedup 1.0000x reference)
"""Trainium2 Bass kernel for a GQA attention layer (B=2, S=2048, D=4096,
32 Q heads / 8 KV heads, rotary, additive mask), SPMD across 8 NeuronCores.

Sharding: core c owns (batch b=c//4, sequence quarter q=c%4) = 512 query
tokens. K/V projections are computed for local tokens only and shared
within each batch's 4 cores via one AllGather. Attention computes
transposed scores (S^T = K^T-tile.T @ Q^T) so exp(S^T) feeds the A*V
matmul directly as the moving operand; softmax normalization is applied
after A*V via a ones-vector column-sum matmul + reciprocal (logits are
bounded, so no max subtraction). The wo projection produces each core's
disjoint 512 output rows, gathered on the host by concatenation.

The additive mask is classified per 128-key chunk across all cores:
all-zero chunks skip the mask add, fully-masked chunks are skipped
entirely, otherwise the (pre-scaled) mask is added on the vector engine.
"""

import os
import sys
from contextlib import ExitStack
from dataclasses import dataclass

import numpy as np

if os.path.isdir("/opt/trn_rl_repo") and "/opt/trn_rl_repo" not in sys.path:
    sys.path.insert(0, "/opt/trn_rl_repo")

import ml_dtypes

import concourse.bass as bass
import concourse.mybir as mybir
import concourse.tile as tile
from concourse import bacc
from concourse.bass_utils import run_bass_kernel_spmd

BF16 = mybir.dt.bfloat16
F32 = mybir.dt.float32
NPBF16 = ml_dtypes.bfloat16
P = 128


@dataclass(frozen=True)
class Cfg:
    S: int = 2048      # full sequence
    D: int = 4096      # model dim
    NH: int = 32       # query heads
    NKV: int = 8       # kv heads
    HD: int = 128      # head dim (must equal P)

    @property
    def T(self):       # local query tokens per core
        return self.S // 4

    @property
    def TS(self):      # 128-token subtiles per local chunk
        return self.T // P

    @property
    def DT(self):      # contraction tiles over D
        return self.D // P

    @property
    def NREP(self):
        return self.NH // self.NKV


FULL = Cfg()


def build_nc(cfg: Cfg, chunk_cls: tuple):
    """chunk_cls[i] for key chunk i (= r*TS+ts): 'v' visible (no mask add),
    'm' mixed (add pre-scaled mask), 's' fully masked (skip chunk)."""
    S, D, NH, NKV, HD = cfg.S, cfg.D, cfg.NH, cfg.NKV, cfg.HD
    T, TS, DT = cfg.T, cfg.TS, cfg.DT
    KVW = NKV * HD               # kv feature width
    NCH = 4 * TS                 # number of 128-token key chunks
    SCALE = float(np.float32(1.0) / np.float32(np.sqrt(np.float32(HD))))
    live = [i for i in range(NCH) if chunk_cls[i] != "s"]
    assert live, "all key chunks masked out"
    has_mask = any(c == "m" for c in chunk_cls)

    nc = bacc.Bacc("TRN2", target_bir_lowering=False, debug=False, num_devices=8)

    xt_d = nc.dram_tensor("xt", [D, T], BF16, kind="ExternalInput")
    wqt_d = nc.dram_tensor("wqt", [D, NH * HD], BF16, kind="ExternalInput")
    wkt_d = nc.dram_tensor("wkt", [D, KVW], BF16, kind="ExternalInput")
    wvt_d = nc.dram_tensor("wvt", [D, KVW], BF16, kind="ExternalInput")
    wot_d = nc.dram_tensor("wot", [NH * HD, D], BF16, kind="ExternalInput")
    cost_d = nc.dram_tensor("cost", [HD, T], F32, kind="ExternalInput")
    sint_d = nc.dram_tensor("sint", [HD, T], F32, kind="ExternalInput")
    maskt_d = nc.dram_tensor("maskt", [S, T], BF16, kind="ExternalInput")
    swap_d = nc.dram_tensor("swapm", [P, P], F32, kind="ExternalInput")
    onesmat_d = nc.dram_tensor("onesmat", [P, P], BF16, kind="ExternalInput")
    out_d = nc.dram_tensor("out", [T, D], F32, kind="ExternalOutput")

    with tile.TileContext(nc) as tc, ExitStack() as ctx:
        persist = ctx.enter_context(tc.tile_pool(name="persist", bufs=1))
        wpool = ctx.enter_context(tc.tile_pool(name="wpool", bufs=3))
        dramp = ctx.enter_context(tc.tile_pool(name="dramp", bufs=1, space="DRAM"))

        # ---- constants ----
        swap_sb = persist.tile([P, P], F32, name="swap_sb")
        nc.sync.dma_start(swap_sb[:], swap_d.ap()[:])
        cost_sb = persist.tile([HD, T], F32, name="cost_sb")
        nc.sync.dma_start(cost_sb[:], cost_d.ap()[:])
        sint_sb = persist.tile([HD, T], F32, name="sint_sb")
        nc.sync.dma_start(sint_sb[:], sint_d.ap()[:])
        onesmat_sb = persist.tile([P, P], BF16, name="onesmat_sb")
        nc.sync.dma_start(onesmat_sb[:], onesmat_d.ap()[:])

        kvin = dramp.tile([2 * KVW, T], BF16, name="kvin")
        kvout = dramp.tile([4 * 2 * KVW, T], BF16, name="kvout")
        kvin_flat = kvin[:].rearrange("a b -> (a b)")
        kvout_flat = kvout[:].rearrange("a b -> (a b)")

        qt = [persist.tile([P, T], BF16, name=f"qt_{h}") for h in range(NH)]

        def rotary(raw_ps, psS, dst_bf16, nm):
            """Interleaved rotary on a [P, T] feature-transposed PSUM tile."""
            raw = rot.tile([P, T], F32, tag="raw", name=f"raw_{nm}")
            nc.scalar.copy(raw[:], raw_ps[:])
            sw_ps = psS.tile([P, T], F32, tag="swp", bufs=2, name=f"swp_{nm}")
            nc.tensor.matmul(sw_ps[:], swap_sb[:], raw[:], start=True, stop=True)
            t1 = rot.tile([P, T], F32, tag="t1", name=f"t1_{nm}")
            nc.vector.tensor_mul(t1[:], raw[:], cost_sb[:])
            t2 = rot.tile([P, T], F32, tag="t2", name=f"t2_{nm}")
            nc.vector.tensor_mul(t2[:], sw_ps[:], sint_sb[:])
            nc.vector.tensor_add(dst_bf16[:], t1[:], t2[:])

        with tc.tile_pool(name="xtp", bufs=1) as xtp, \
             tc.tile_pool(name="rot", bufs=2) as rot, \
             tc.tile_pool(name="psP", bufs=1, space="PSUM") as psP:
            xt_sb = []
            for d in range(DT):
                t = xtp.tile([P, T], BF16, name=f"xt_sb_{d}")
                nc.sync.dma_start(t[:], xt_d.ap()[d * P:(d + 1) * P, :])
                xt_sb.append(t)

            # ---- K^T projection (local tokens) + rotary ----
            ktloc = [xtp.tile([P, T], BF16, name=f"ktloc_{kvh}")
                     for kvh in range(NKV)]
            for g in range(NKV // 4):
                kps = [psP.tile([P, T], F32, tag=f"pj{j}", name=f"kps_{g}_{j}")
                       for j in range(4)]
                for d in range(DT):
                    wrow = wpool.tile([P, 512], BF16, tag="wkv", bufs=4, name=f"wk_{g}_{d}")
                    nc.sync.dma_start(
                        wrow[:], wkt_d.ap()[d * P:(d + 1) * P, g * 512:(g + 1) * 512])
                    for j in range(4):
                        nc.tensor.matmul(
                            kps[j][:], wrow[:, j * HD:(j + 1) * HD], xt_sb[d][:],
                            start=(d == 0), stop=(d == DT - 1))
                for j in range(4):
                    kvh = g * 4 + j
                    rotary(kps[j], psP, ktloc[kvh], f"k{kvh}")

            # ---- V projection (local tokens), [token, feature] layout ----
            vtloc = [xtp.tile([P, KVW], BF16, name=f"vtloc_{ts}")
                     for ts in range(TS)]
            for eh in range(KVW // 512):
                vps = [psP.tile([P, 512], F32, tag=f"pj{j}", name=f"vps_{eh}_{j}")
                       for j in range(TS)]
                for d in range(DT):
                    wrow = wpool.tile([P, 512], BF16, tag="wkv", bufs=4, name=f"wv_{eh}_{d}")
                    nc.sync.dma_start(
                        wrow[:], wvt_d.ap()[d * P:(d + 1) * P, eh * 512:(eh + 1) * 512])
                    for ts in range(TS):
                        nc.tensor.matmul(
                            vps[ts][:], xt_sb[d][:, ts * P:(ts + 1) * P], wrow[:],
                            start=(d == 0), stop=(d == DT - 1))
                for ts in range(TS):
                    nc.scalar.copy(vtloc[ts][:, eh * 512:(eh + 1) * 512], vps[ts][:])

            # ---- pack K^T and V into the collective input buffer ----
            for kvh in range(NKV):
                nc.sync.dma_start(kvin[kvh * HD:(kvh + 1) * HD, :], ktloc[kvh][:])
            vbase = KVW * T
            for ts in range(TS):
                nc.sync.dma_start(
                    kvin_flat[vbase + ts * P * KVW: vbase + (ts + 1) * P * KVW]
                    .rearrange("(p f) -> p f", p=P),
                    vtloc[ts][:])

            nc.gpsimd.collective_compute(
                "AllGather",
                mybir.AluOpType.bypass,
                replica_groups=[[0, 1, 2, 3], [4, 5, 6, 7]],
                ins=[kvin[:].opt()],
                outs=[kvout[:].opt()],
            )

            # ---- Q^T projection + rotary (overlaps the AllGather) ----
            for g in range(NH // 4):
                qps = [psP.tile([P, T], F32, tag=f"pj{j}", name=f"qps_{g}_{j}")
                       for j in range(4)]
                for d in range(DT):
                    wrow = wpool.tile([P, 512], BF16, tag="wq", bufs=4, name=f"wq_{g}_{d}")
                    nc.sync.dma_start(
                        wrow[:], wqt_d.ap()[d * P:(d + 1) * P, g * 512:(g + 1) * 512])
                    for j in range(4):
                        nc.tensor.matmul(
                            qps[j][:], wrow[:, j * HD:(j + 1) * HD], xt_sb[d][:],
                            start=(d == 0), stop=(d == DT - 1))
                for j in range(4):
                    h = g * 4 + j
                    rotary(qps[j], psP, qt[h], f"q{h}")

        maskt_sb = {}
        for i in range(NCH):
            if chunk_cls[i] == "m":
                t = persist.tile([P, T], BF16, name=f"maskt_sb_{i}")
                nc.sync.dma_start(t[:], maskt_d.ap()[i * P:(i + 1) * P, :])
                maskt_sb[i] = t

        # ---- attention, streaming gathered K^T / V per kv head ----
        att = [persist.tile([P, T], BF16, name=f"att_{h}") for h in range(NH)]
        with tc.tile_pool(name="kvp", bufs=1) as kvp, \
             tc.tile_pool(name="atw", bufs=1) as work, \
             tc.tile_pool(name="psA", bufs=1, space="PSUM") as psA:
            for kvh in range(NKV):
                ktl = {}
                vtl = {}
                for r in range(4):
                    kt_t = kvp.tile([P, T], BF16, tag="kt", bufs=8,
                                    name=f"kt_{kvh}_{r}")
                    nc.sync.dma_start(
                        kt_t[:],
                        kvout[r * 2 * KVW + kvh * HD: r * 2 * KVW + (kvh + 1) * HD, :])
                    ktl[r] = kt_t
                    vchunk = kvout_flat[(r * 2 * KVW + KVW) * T:
                                        (r * 2 * KVW + 2 * KVW) * T] \
                        .rearrange("(t f) -> t f", f=KVW)
                    for ts in range(TS):
                        i = r * TS + ts
                        if chunk_cls[i] == "s":
                            continue
                        vt_t = kvp.tile([P, HD], BF16, tag="vts", bufs=2 * NCH,
                                        name=f"vt_{kvh}_{r}_{ts}")
                        nc.sync.dma_start(
                            vt_t[:],
                            vchunk[ts * P:(ts + 1) * P, kvh * HD:(kvh + 1) * HD])
                        vtl[i] = vt_t
                # Paired key chunks ([128,1024] score tiles, one exp per
                # pair halves ACT fixed cost); two query heads interleaved
                # per sweep keep the PE stream dense. Z (softmax denominator,
                # broadcast over partitions) accumulates on the PE from the
                # bf16 exp tiles via an all-ones stationary operand.
                pairs = [live[k:k + 2] for k in range(0, len(live), 2)]
                for sub in range(cfg.NREP // 2):
                    hs = [kvh * cfg.NREP + sub * 2, kvh * cfg.NREP + sub * 2 + 1]
                    av_ps = {}
                    zb_ps = {}
                    for j, h in enumerate(hs):
                        av_ps[j] = psA.tile([P, T], F32, tag="av", bufs=2,
                                            name=f"av_{h}")
                        zb_ps[j] = psA.tile([P, T], F32, tag="zb", bufs=2,
                                            name=f"zb_{h}")
                    for pi, pr in enumerate(pairs):
                        for j, h in enumerate(hs):
                            s2 = psA.tile([P, 2 * T], F32, tag="s", bufs=2,
                                          name=f"s_{h}_{pi}")
                            for m, i in enumerate(pr):
                                r, ts = i // TS, i % TS
                                nc.tensor.matmul(
                                    s2[:, m * T:(m + 1) * T],
                                    ktl[r][:, ts * P:(ts + 1) * P], qt[h][:],
                                    start=True, stop=True)
                                if chunk_cls[i] == "m":
                                    nc.vector.tensor_add(
                                        s2[:, m * T:(m + 1) * T],
                                        s2[:, m * T:(m + 1) * T],
                                        maskt_sb[i][:])
                            w2 = len(pr) * T
                            e2 = work.tile([P, 2 * T], BF16, tag="et", bufs=6,
                                           name=f"e_{h}_{pi}")
                            nc.scalar.activation(
                                e2[:, :w2], s2[:, :w2],
                                mybir.ActivationFunctionType.Exp, scale=SCALE)
                            for m, i in enumerate(pr):
                                nc.tensor.matmul(
                                    av_ps[j][:], vtl[i][:],
                                    e2[:, m * T:(m + 1) * T],
                                    start=(i == live[0]), stop=(i == live[-1]))
                                nc.tensor.matmul(
                                    zb_ps[j][:], onesmat_sb[:],
                                    e2[:, m * T:(m + 1) * T],
                                    start=(i == live[0]), stop=(i == live[-1]))
                    for j, h in enumerate(hs):
                        rzb = work.tile([P, T], F32, tag="rzbs", bufs=2,
                                        name=f"rzbs_{h}")
                        nc.vector.reciprocal_approx_fast(out=rzb[:],
                                                         in_=zb_ps[j][:])
                        nc.vector.tensor_mul(att[h][:], av_ps[j][:], rzb[:])

        # ---- output projection ----
        with tc.tile_pool(name="osbp", bufs=1) as osbp, \
             tc.tile_pool(name="psW", bufs=1, space="PSUM") as psW:
            for douth in range(D // 512):
                ops = [psW.tile([P, 512], F32, tag=f"pw{tt}", bufs=2,
                                name=f"ops_{douth}_{tt}") for tt in range(TS)]
                for e in range(NH):
                    wrow = wpool.tile([P, 512], BF16, tag="wo", bufs=6, name=f"wo_{douth}_{e}")
                    nc.sync.dma_start(
                        wrow[:],
                        wot_d.ap()[e * P:(e + 1) * P, douth * 512:(douth + 1) * 512])
                    for tt in range(TS):
                        nc.tensor.matmul(
                            ops[tt][:], att[e][:, tt * P:(tt + 1) * P], wrow[:],
                            start=(e == 0), stop=(e == NH - 1))
                for tt in range(TS):
                    osb = osbp.tile([P, 512], F32, tag="osb", bufs=4,
                                    name=f"osb_{douth}_{tt}")
                    nc.scalar.copy(osb[:], ops[tt][:])
                    nc.sync.dma_start(
                        out_d.ap()[tt * P:(tt + 1) * P, douth * 512:(douth + 1) * 512],
                        osb[:])

    nc.compile()
    return nc


def classify_chunks(mask, cfg: Cfg):
    """Per 128-key chunk, over all cores' query slices: 'v' if mask is all
    zero, 's' if all <= -1e8 (exp underflows for any realistic logit),
    'm' otherwise."""
    S, T, TS = cfg.S, cfg.T, cfg.TS
    NCH = 4 * TS
    cls = []
    for i in range(NCH):
        rows = mask[:, i * P:(i + 1) * P]      # [S queries, 128 keys]
        if not rows.any():
            cls.append("v")
        elif (rows <= -1e8).all():
            cls.append("s")
        else:
            cls.append("m")
    if all(c == "s" for c in cls):
        cls = ["m"] * NCH
    return tuple(cls)


def make_in_maps(x, freqs_cis, mask, wq, wk, wv, wo, cfg: Cfg):
    S, D, T, HD = cfg.S, cfg.D, cfg.T, cfg.HD
    SCALE = np.float32(1.0) / np.float32(np.sqrt(np.float32(HD)))
    x = np.asarray(x, np.float32)
    fc = np.asarray(freqs_cis, np.float32)
    mask = np.asarray(mask, np.float32)
    wqt = np.ascontiguousarray(np.asarray(wq, np.float32).T).astype(NPBF16)
    wkt = np.ascontiguousarray(np.asarray(wk, np.float32).T).astype(NPBF16)
    wvt = np.ascontiguousarray(np.asarray(wv, np.float32).T).astype(NPBF16)
    wot = np.ascontiguousarray(np.asarray(wo, np.float32).T).astype(NPBF16)

    swapm = np.zeros((P, P), np.float32)
    for i in range(P // 2):
        swapm[2 * i, 2 * i + 1] = 1.0
        swapm[2 * i + 1, 2 * i] = 1.0
    onesmat = np.ones((P, P), NPBF16)

    in_maps = []
    for c in range(8):
        b, q = c // 4, c % 4
        sl = slice(T * q, T * (q + 1))
        xt = np.ascontiguousarray(x[b, sl, :].T).astype(NPBF16)
        cost = np.repeat(fc[sl, :, 0].T, 2, axis=0).astype(np.float32)
        sint = np.repeat(fc[sl, :, 1].T, 2, axis=0).astype(np.float32)
        sint[0::2, :] *= -1.0
        # pre-divide by the score scale so the on-device op is a plain add
        maskt = np.ascontiguousarray((mask[sl, :] / SCALE).T).astype(NPBF16)
        in_maps.append({
            "xt": xt, "wqt": wqt, "wkt": wkt, "wvt": wvt, "wot": wot,
            "cost": np.ascontiguousarray(cost),
            "sint": np.ascontiguousarray(sint),
            "maskt": maskt, "swapm": swapm, "onesmat": onesmat,
        })
    return in_maps


_NC_CACHE = {}


def kernel_run(x, start_pos, freqs_cis, mask, wq, wk, wv, wo,
               cfg: Cfg = FULL, trace=False):
    chunk_cls = classify_chunks(np.asarray(mask, np.float32), cfg)
    in_maps = make_in_maps(x, freqs_cis, mask, wq, wk, wv, wo, cfg)
    key = (cfg, chunk_cls)
    if key not in _NC_CACHE:
        _NC_CACHE[key] = build_nc(cfg, chunk_cls)
    nc = _NC_CACHE[key]
    res = run_bass_kernel_spmd(nc, in_maps, core_ids=list(range(8)), trace=trace)
    outs = np.stack([res.results[c]["out"] for c in range(8)])
    full = outs.reshape(2, 4, cfg.T, cfg.D).reshape(2, cfg.S, cfg.D)
    return full, res


def kernel(x, start_pos=None, freqs_cis=None, mask=None, wq=None, wk=None,
           wv=None, wo=None):
    full, _ = kernel_run(x, start_pos, freqs_cis, mask, wq, wk, wv, wo)
    return full


# revision 12
# speedup vs baseline: 1.1826x; 1.1826x over previous
"""Trainium2 Bass kernel for a GQA attention layer (B=2, S=2048, D=4096,
32 Q heads / 8 KV heads, rotary, additive mask), SPMD across 8 NeuronCores.

Sharding: core c owns (batch b=c//4, sequence quarter q=c%4) = 512 query
tokens. K/V projections are computed for local tokens only and shared
within each batch's 4 cores via one AllGather. Attention computes
transposed scores (S^T = K^T-tile.T @ Q^T) so exp(S^T) feeds the A*V
matmul directly as the moving operand; softmax normalization is applied
after A*V via a ones-vector column-sum matmul + reciprocal (logits are
bounded, so no max subtraction). The wo projection produces each core's
disjoint 512 output rows, gathered on the host by concatenation.

The additive mask is classified per 128-key chunk across all cores:
all-zero chunks skip the mask add, fully-masked chunks are skipped
entirely, otherwise the (pre-scaled) mask is added on the vector engine.
"""

import os
import sys
from contextlib import ExitStack
from dataclasses import dataclass

import numpy as np

if os.path.isdir("/opt/trn_rl_repo") and "/opt/trn_rl_repo" not in sys.path:
    sys.path.insert(0, "/opt/trn_rl_repo")

import ml_dtypes

import concourse.bass as bass
import concourse.mybir as mybir
import concourse.tile as tile
from concourse import bacc
from concourse.bass_utils import run_bass_kernel_spmd

BF16 = mybir.dt.bfloat16
F32 = mybir.dt.float32
NPBF16 = ml_dtypes.bfloat16
P = 128


@dataclass(frozen=True)
class Cfg:
    S: int = 2048      # full sequence
    D: int = 4096      # model dim
    NH: int = 32       # query heads
    NKV: int = 8       # kv heads
    HD: int = 128      # head dim (must equal P)

    @property
    def T(self):       # local query tokens per core
        return self.S // 4

    @property
    def TS(self):      # 128-token subtiles per local chunk
        return self.T // P

    @property
    def DT(self):      # contraction tiles over D
        return self.D // P

    @property
    def NREP(self):
        return self.NH // self.NKV


FULL = Cfg()


def build_nc(cfg: Cfg, chunk_cls: tuple):
    """chunk_cls[i] for key chunk i (= r*TS+ts): 'v' visible (no mask add),
    'm' mixed (add pre-scaled mask), 's' fully masked (skip chunk)."""
    S, D, NH, NKV, HD = cfg.S, cfg.D, cfg.NH, cfg.NKV, cfg.HD
    T, TS, DT = cfg.T, cfg.TS, cfg.DT
    KVW = NKV * HD               # kv feature width
    NCH = 4 * TS                 # number of 128-token key chunks
    SCALE = float(np.float32(1.0) / np.float32(np.sqrt(np.float32(HD))))
    live = [i for i in range(NCH) if chunk_cls[i] != "s"]
    assert live, "all key chunks masked out"
    has_mask = any(c == "m" for c in chunk_cls)

    nc = bacc.Bacc("TRN2", target_bir_lowering=False, debug=False, num_devices=8)

    xt_d = nc.dram_tensor("xt", [D, T], BF16, kind="ExternalInput")
    wqt_d = nc.dram_tensor("wqt", [D, NH * HD], BF16, kind="ExternalInput")
    wkt_d = nc.dram_tensor("wkt", [D, KVW], BF16, kind="ExternalInput")
    wvt_d = nc.dram_tensor("wvt", [D, KVW], BF16, kind="ExternalInput")
    wot_d = nc.dram_tensor("wot", [NH * HD, D], BF16, kind="ExternalInput")
    cost_d = nc.dram_tensor("cost", [HD, T], F32, kind="ExternalInput")
    sint_d = nc.dram_tensor("sint", [HD, T], F32, kind="ExternalInput")
    maskt_d = nc.dram_tensor("maskt", [S, T], BF16, kind="ExternalInput")
    swap_d = nc.dram_tensor("swapm", [P, P], BF16, kind="ExternalInput")
    onesmat_d = nc.dram_tensor("onesmat", [P, P], BF16, kind="ExternalInput")
    out_d = nc.dram_tensor("out", [T, D], F32, kind="ExternalOutput")

    with tile.TileContext(nc) as tc, ExitStack() as ctx:
        persist = ctx.enter_context(tc.tile_pool(name="persist", bufs=1))
        wpool = ctx.enter_context(tc.tile_pool(name="wpool", bufs=3))
        dramp = ctx.enter_context(tc.tile_pool(name="dramp", bufs=1, space="DRAM"))

        # ---- constants ----
        swap_sb = persist.tile([P, P], BF16, name="swap_sb")
        nc.sync.dma_start(swap_sb[:], swap_d.ap()[:])
        cost_sb = persist.tile([HD, T], F32, name="cost_sb")
        nc.sync.dma_start(cost_sb[:], cost_d.ap()[:])
        sint_sb = persist.tile([HD, T], F32, name="sint_sb")
        nc.sync.dma_start(sint_sb[:], sint_d.ap()[:])
        onesmat_sb = persist.tile([P, P], BF16, name="onesmat_sb")
        nc.sync.dma_start(onesmat_sb[:], onesmat_d.ap()[:])

        kvin = dramp.tile([2 * KVW, T], BF16, name="kvin")
        kvout = dramp.tile([4 * 2 * KVW, T], BF16, name="kvout")
        kvin_flat = kvin[:].rearrange("a b -> (a b)")
        kvout_flat = kvout[:].rearrange("a b -> (a b)")

        qt = [persist.tile([P, T], BF16, name=f"qt_{h}") for h in range(NH)]

        def rotary(raw_ps, psS, dst_bf16, nm):
            """Interleaved rotary on a [P, T] feature-transposed PSUM tile."""
            raw = rot.tile([P, T], BF16, tag="raw", name=f"raw_{nm}")
            nc.scalar.copy(raw[:], raw_ps[:])
            sw_ps = psS.tile([P, T], F32, tag="swp", bufs=2, name=f"swp_{nm}")
            nc.tensor.matmul(sw_ps[:], swap_sb[:], raw[:], start=True, stop=True)
            t1 = rot.tile([P, T], F32, tag="t1", name=f"t1_{nm}")
            nc.vector.tensor_mul(t1[:], raw[:], cost_sb[:])
            t2 = rot.tile([P, T], F32, tag="t2", name=f"t2_{nm}")
            nc.vector.tensor_mul(t2[:], sw_ps[:], sint_sb[:])
            nc.vector.tensor_add(dst_bf16[:], t1[:], t2[:])

        with tc.tile_pool(name="xtp", bufs=1) as xtp, \
             tc.tile_pool(name="rot", bufs=2) as rot, \
             tc.tile_pool(name="psP", bufs=1, space="PSUM") as psP:
            xt_sb = []
            for d in range(DT):
                t = xtp.tile([P, T], BF16, name=f"xt_sb_{d}")
                xt_sb.append(t)
            xt_loaded = [False] * DT

            def load_xt(d):
                if not xt_loaded[d]:
                    nc.sync.dma_start(xt_sb[d][:],
                                      xt_d.ap()[d * P:(d + 1) * P, :])
                    xt_loaded[d] = True

            # ---- K^T projection (local tokens) + rotary ----
            ktloc = [xtp.tile([P, T], BF16, name=f"ktloc_{kvh}")
                     for kvh in range(NKV)]
            for g in range(NKV // 4):
                kps = [psP.tile([P, T], F32, tag=f"pj{j}", name=f"kps_{g}_{j}")
                       for j in range(4)]
                for d in range(DT):
                    wrow = wpool.tile([P, 512], BF16, tag="wkv", bufs=4, name=f"wk_{g}_{d}")
                    nc.sync.dma_start(
                        wrow[:], wkt_d.ap()[d * P:(d + 1) * P, g * 512:(g + 1) * 512])
                    load_xt(d)
                    for j in range(4):
                        nc.tensor.matmul(
                            kps[j][:], wrow[:, j * HD:(j + 1) * HD], xt_sb[d][:],
                            start=(d == 0), stop=(d == DT - 1))
                for j in range(4):
                    kvh = g * 4 + j
                    rotary(kps[j], psP, ktloc[kvh], f"k{kvh}")

            # ---- V projection (local tokens), [token, feature] layout ----
            vtloc = [xtp.tile([P, KVW], BF16, name=f"vtloc_{ts}")
                     for ts in range(TS)]
            for eh in range(KVW // 512):
                vps = [psP.tile([P, 512], F32, tag=f"pj{j}", name=f"vps_{eh}_{j}")
                       for j in range(TS)]
                for d in range(DT):
                    wrow = wpool.tile([P, 512], BF16, tag="wkv", bufs=4, name=f"wv_{eh}_{d}")
                    nc.sync.dma_start(
                        wrow[:], wvt_d.ap()[d * P:(d + 1) * P, eh * 512:(eh + 1) * 512])
                    for ts in range(TS):
                        nc.tensor.matmul(
                            vps[ts][:], xt_sb[d][:, ts * P:(ts + 1) * P], wrow[:],
                            start=(d == 0), stop=(d == DT - 1))
                for ts in range(TS):
                    nc.scalar.copy(vtloc[ts][:, eh * 512:(eh + 1) * 512], vps[ts][:])

            # ---- pack K^T and V into the collective input buffer ----
            for kvh in range(NKV):
                nc.sync.dma_start(kvin[kvh * HD:(kvh + 1) * HD, :], ktloc[kvh][:])
            vbase = KVW * T
            for ts in range(TS):
                nc.sync.dma_start(
                    kvin_flat[vbase + ts * P * KVW: vbase + (ts + 1) * P * KVW]
                    .rearrange("(p f) -> p f", p=P),
                    vtloc[ts][:])

            nc.gpsimd.collective_compute(
                "AllGather",
                mybir.AluOpType.bypass,
                replica_groups=[[0, 1, 2, 3], [4, 5, 6, 7]],
                ins=[kvin[:].opt()],
                outs=[kvout[:].opt()],
            )

            # ---- Q^T projection + rotary (overlaps the AllGather) ----
            for g in range(NH // 4):
                qps = [psP.tile([P, T], F32, tag=f"pj{j}", name=f"qps_{g}_{j}")
                       for j in range(4)]
                for d in range(DT):
                    wrow = wpool.tile([P, 512], BF16, tag="wq", bufs=4, name=f"wq_{g}_{d}")
                    nc.sync.dma_start(
                        wrow[:], wqt_d.ap()[d * P:(d + 1) * P, g * 512:(g + 1) * 512])
                    for j in range(4):
                        nc.tensor.matmul(
                            qps[j][:], wrow[:, j * HD:(j + 1) * HD], xt_sb[d][:],
                            start=(d == 0), stop=(d == DT - 1))
                for j in range(4):
                    h = g * 4 + j
                    rotary(qps[j], psP, qt[h], f"q{h}")

        tc.no_sync_barrier()

        maskt_sb = {}
        for i in range(NCH):
            if chunk_cls[i] == "m":
                t = persist.tile([P, T], BF16, name=f"maskt_sb_{i}")
                nc.sync.dma_start(t[:], maskt_d.ap()[i * P:(i + 1) * P, :])
                maskt_sb[i] = t

        # ---- attention, streaming gathered K^T / V per kv head ----
        att = [persist.tile([P, T], BF16, name=f"att_{h}") for h in range(NH)]
        with tc.tile_pool(name="kvp", bufs=1) as kvp, \
             tc.tile_pool(name="atw", bufs=1) as work, \
             tc.tile_pool(name="psA", bufs=1, space="PSUM") as psA:
            for kvh in range(NKV):
                ktl = {}
                vtl = {}
                for r in range(4):
                    kt_t = kvp.tile([P, T], BF16, tag="kt", bufs=8,
                                    name=f"kt_{kvh}_{r}")
                    nc.sync.dma_start(
                        kt_t[:],
                        kvout[r * 2 * KVW + kvh * HD: r * 2 * KVW + (kvh + 1) * HD, :])
                    ktl[r] = kt_t
                    vchunk = kvout_flat[(r * 2 * KVW + KVW) * T:
                                        (r * 2 * KVW + 2 * KVW) * T] \
                        .rearrange("(t f) -> t f", f=KVW)
                    for ts in range(TS):
                        i = r * TS + ts
                        if chunk_cls[i] == "s":
                            continue
                        vt_t = kvp.tile([P, HD], BF16, tag="vts", bufs=2 * NCH,
                                        name=f"vt_{kvh}_{r}_{ts}")
                        nc.sync.dma_start(
                            vt_t[:],
                            vchunk[ts * P:(ts + 1) * P, kvh * HD:(kvh + 1) * HD])
                        vtl[i] = vt_t
                # Paired key chunks ([128,1024] score tiles, one exp per
                # pair halves ACT fixed cost); two query heads interleaved
                # per sweep keep the PE stream dense. Z (softmax denominator,
                # broadcast over partitions) accumulates on the PE from the
                # bf16 exp tiles via an all-ones stationary operand.
                pairs = [live[k:k + 2] for k in range(0, len(live), 2)]
                for sub in range(cfg.NREP // 2):
                    hs = [kvh * cfg.NREP + sub * 2, kvh * cfg.NREP + sub * 2 + 1]
                    av_ps = {}
                    zb_ps = {}
                    for j, h in enumerate(hs):
                        av_ps[j] = psA.tile([P, T], F32, tag="av", bufs=2,
                                            name=f"av_{h}")
                        zb_ps[j] = psA.tile([P, T], F32, tag="zb", bufs=2,
                                            name=f"zb_{h}")
                    for pi, pr in enumerate(pairs):
                        for j, h in enumerate(hs):
                            s2 = psA.tile([P, 2 * T], F32, tag="s", bufs=2,
                                          name=f"s_{h}_{pi}")
                            for m, i in enumerate(pr):
                                r, ts = i // TS, i % TS
                                nc.tensor.matmul(
                                    s2[:, m * T:(m + 1) * T],
                                    ktl[r][:, ts * P:(ts + 1) * P], qt[h][:],
                                    start=True, stop=True)
                                if chunk_cls[i] == "m":
                                    nc.vector.tensor_add(
                                        s2[:, m * T:(m + 1) * T],
                                        s2[:, m * T:(m + 1) * T],
                                        maskt_sb[i][:])
                            w2 = len(pr) * T
                            e2 = work.tile([P, 2 * T], BF16, tag="et", bufs=6,
                                           name=f"e_{h}_{pi}")
                            nc.scalar.activation(
                                e2[:, :w2], s2[:, :w2],
                                mybir.ActivationFunctionType.Exp, scale=SCALE)
                            for m, i in enumerate(pr):
                                nc.tensor.matmul(
                                    av_ps[j][:], vtl[i][:],
                                    e2[:, m * T:(m + 1) * T],
                                    start=(i == live[0]), stop=(i == live[-1]))
                                nc.tensor.matmul(
                                    zb_ps[j][:], onesmat_sb[:],
                                    e2[:, m * T:(m + 1) * T],
                                    start=(i == live[0]), stop=(i == live[-1]))
                    for j, h in enumerate(hs):
                        rzb = work.tile([P, T], F32, tag="rzbs", bufs=2,
                                        name=f"rzbs_{h}")
                        nc.vector.reciprocal_approx_fast(out=rzb[:],
                                                         in_=zb_ps[j][:])
                        nc.vector.tensor_mul(att[h][:], av_ps[j][:], rzb[:])

        tc.no_sync_barrier()

        # ---- output projection ----
        with tc.tile_pool(name="osbp", bufs=1) as osbp, \
             tc.tile_pool(name="psW", bufs=1, space="PSUM") as psW:
            for douth in range(D // 512):
                ops = [psW.tile([P, 512], F32, tag=f"pw{tt}", bufs=2,
                                name=f"ops_{douth}_{tt}") for tt in range(TS)]
                for e in range(NH):
                    wrow = wpool.tile([P, 512], BF16, tag="wo", bufs=6, name=f"wo_{douth}_{e}")
                    nc.sync.dma_start(
                        wrow[:],
                        wot_d.ap()[e * P:(e + 1) * P, douth * 512:(douth + 1) * 512])
                    for tt in range(TS):
                        nc.tensor.matmul(
                            ops[tt][:], att[e][:, tt * P:(tt + 1) * P], wrow[:],
                            start=(e == 0), stop=(e == NH - 1))
                for tt in range(TS):
                    osb = osbp.tile([P, 512], F32, tag="osb", bufs=4,
                                    name=f"osb_{douth}_{tt}")
                    nc.scalar.copy(osb[:], ops[tt][:])
                    nc.sync.dma_start(
                        out_d.ap()[tt * P:(tt + 1) * P, douth * 512:(douth + 1) * 512],
                        osb[:])

    nc.compile()
    return nc


def classify_chunks(mask, cfg: Cfg):
    """Per 128-key chunk, over all cores' query slices: 'v' if mask is all
    zero, 's' if all <= -1e8 (exp underflows for any realistic logit),
    'm' otherwise."""
    S, T, TS = cfg.S, cfg.T, cfg.TS
    NCH = 4 * TS
    cls = []
    for i in range(NCH):
        rows = mask[:, i * P:(i + 1) * P]      # [S queries, 128 keys]
        if not rows.any():
            cls.append("v")
        elif (rows <= -1e8).all():
            cls.append("s")
        else:
            cls.append("m")
    if all(c == "s" for c in cls):
        cls = ["m"] * NCH
    return tuple(cls)


def make_in_maps(x, freqs_cis, mask, wq, wk, wv, wo, cfg: Cfg):
    S, D, T, HD = cfg.S, cfg.D, cfg.T, cfg.HD
    SCALE = np.float32(1.0) / np.float32(np.sqrt(np.float32(HD)))
    x = np.asarray(x, np.float32)
    fc = np.asarray(freqs_cis, np.float32)
    mask = np.asarray(mask, np.float32)
    wqt = np.ascontiguousarray(np.asarray(wq, np.float32).T).astype(NPBF16)
    wkt = np.ascontiguousarray(np.asarray(wk, np.float32).T).astype(NPBF16)
    wvt = np.ascontiguousarray(np.asarray(wv, np.float32).T).astype(NPBF16)
    wot = np.ascontiguousarray(np.asarray(wo, np.float32).T).astype(NPBF16)

    swapm = np.zeros((P, P), np.float32)
    for i in range(P // 2):
        swapm[2 * i, 2 * i + 1] = 1.0
        swapm[2 * i + 1, 2 * i] = 1.0
    swapm = swapm.astype(NPBF16)
    onesmat = np.ones((P, P), NPBF16)

    in_maps = []
    for c in range(8):
        b, q = c // 4, c % 4
        sl = slice(T * q, T * (q + 1))
        xt = np.ascontiguousarray(x[b, sl, :].T).astype(NPBF16)
        cost = np.repeat(fc[sl, :, 0].T, 2, axis=0).astype(np.float32)
        sint = np.repeat(fc[sl, :, 1].T, 2, axis=0).astype(np.float32)
        sint[0::2, :] *= -1.0
        # pre-divide by the score scale so the on-device op is a plain add
        maskt = np.ascontiguousarray((mask[sl, :] / SCALE).T).astype(NPBF16)
        in_maps.append({
            "xt": xt, "wqt": wqt, "wkt": wkt, "wvt": wvt, "wot": wot,
            "cost": np.ascontiguousarray(cost),
            "sint": np.ascontiguousarray(sint),
            "maskt": maskt, "swapm": swapm, "onesmat": onesmat,
        })
    return in_maps


_NC_CACHE = {}


def kernel_run(x, start_pos, freqs_cis, mask, wq, wk, wv, wo,
               cfg: Cfg = FULL, trace=False):
    chunk_cls = classify_chunks(np.asarray(mask, np.float32), cfg)
    in_maps = make_in_maps(x, freqs_cis, mask, wq, wk, wv, wo, cfg)
    key = (cfg, chunk_cls)
    if key not in _NC_CACHE:
        _NC_CACHE[key] = build_nc(cfg, chunk_cls)
    nc = _NC_CACHE[key]
    res = run_bass_kernel_spmd(nc, in_maps, core_ids=list(range(8)), trace=trace)
    outs = np.stack([res.results[c]["out"] for c in range(8)])
    full = outs.reshape(2, 4, cfg.T, cfg.D).reshape(2, cfg.S, cfg.D)
    return full, res


def kernel(x, start_pos=None, freqs_cis=None, mask=None, wq=None, wk=None,
           wv=None, wo=None):
    full, _ = kernel_run(x, start_pos, freqs_cis, mask, wq, wk, wv, wo)
    return full
